# revision 71
# baseline (speedup 1.0000x reference)
"""Trainium2 Bass kernel for a single-head transformer decoder layer.

Model (per batch element, S=2048, E=1024, F=4096):
    xn  = LN(tgt);  sa = causal_attn(xn)       ; h1  = tgt + sa
    xn2 = LN(h1);   ca = cross_attn(xn2, src)  ; h2  = h1 + ca
    xn3 = LN(h2);   ff = relu(xn3@W1.T)@W2.T   ; out = h2 + ff

Sharding: 8 cores = 4 batches x 2-way query-row split.  Core c owns batch
b=c//2 and interleaved 128-row chunks g = 2*j + (c%2), j=0..7 (zig-zag, so
causal-attention work is balanced across the pair).  K/V projections over
all 2048 rows are duplicated within each pair; no collectives.

On-chip layout: activations are stored transposed [feature(part), row(free)],
which lets every matmul in the layer run without any on-chip transpose:
  - proj:    out_T[f,r]   = mm(lhsT=W_T[e,f] blk,  rhs=x_T[e,r])
  - V:       V_nat[r,v]   = mm(lhsT=x_T[e,r] blk,  rhs=W_T[e,v])
  - scores:  s_T[kr,qr]   = mm(lhsT=K_T[e,kr] blk, rhs=Q_T[e,qr])
  - softmax: et = exp(s/sqrt(E)) via ACT scale (no max-sub needed; scores
    are O(1)), causal mask as additive bf16 input data
  - AV:      a_T[af,qr]   = mm(lhsT=V_nat[kr,af] blk, rhs=expT[kr,qr])
  - the AV numerator is divided by the softmax denominator BEFORE the fp8
    store (DVE mul with the broadcast reciprocal), so attn values are
    O(|V|) for every row -- early causal rows with tiny denominators would
    otherwise drown in the fp8 subnormal floor.
LN over the feature dim (= partitions) uses ones-matmul column sums; the
[128,x] stationary ones matrix makes the stats land pre-broadcast across
partitions.  The residual stream (tgto/h1/h2) is bf16, so the stats
matmuls read it directly with no staging copies.

fp8: the attention path runs fp8e4 (e4m3, +/-240) in DoubleRow perf mode
(2 k-tiles per pass = 2x bf16 matmul throughput).  Weights are scaled by
WS=256 on the host so their ~0.02-sigma values sit in fp8's normal range;
the 1/WS is folded into the (already needed) PSUM-evacuation scale.  The
FFN stays bf16: the decoder's large output values come from the FFN, so
fp8 there costs ~10x the end-to-end error of the attention path
(measured).  The first owned 128 q-rows of SA (<=256 keys, no softmax
averaging of fp8 noise) are recomputed in clean bf16 and overwrite h1.

Dataflow is software-pipelined for tensor-engine occupancy: the serial
Q1 chain is emitted first, K2/V2 (independent of the whole tgt path) is
emitted late so it acts as low-priority gap filler everywhere, the head
patch's long independent chain sits before SA, and each attention's t=0
half flows into O-proj/LN/next-Q emission before the t=1 half so the
serial LN chains overlap attention matmuls.

Measured end-to-end max-rel-err ~7e-3 vs the fp32 reference (gate 2e-2).
"""

import os
import sys

import numpy as np

for _p in ("/opt/trn_rl_repo", "/root/.axon_site/_ro/trn_rl_repo"):
    if os.path.isdir(_p) and _p not in sys.path:
        sys.path.insert(0, _p)

import ml_dtypes  # noqa: E402

import concourse.bass as bass  # noqa: E402
import concourse.tile as tile  # noqa: E402
from concourse import bacc, mybir  # noqa: E402
from concourse.bass_utils import run_bass_kernel_spmd  # noqa: E402

E = 1024
S = 2048
B = 4
F = 4096
P = 128
NE = E // P          # 8 feature blocks
NF = F // P          # 32 ff blocks
NKB = S // P         # 16 key-row blocks
RO = 1024            # owned query rows per core
NCORES = 8

F32 = mybir.dt.float32
BF16 = mybir.dt.bfloat16
FP8 = mybir.dt.float8e4
BF = ml_dtypes.bfloat16
F8 = ml_dtypes.float8_e4m3
ALU = mybir.AluOpType
ACT_F = mybir.ActivationFunctionType
DR = mybir.MatmulPerfMode.DoubleRow

NEG = -1e30
WS = 256.0           # host-side weight scale (fp8 subnormal avoidance)
IWS = 1.0 / WS
ISQ = 1.0 / 32.0     # 1/sqrt(E)
SM = 512.0           # host scale for composed E x E matrices (Mh, Mvo);
                     # their 25-sigma outliers clip above ~750
ZS = 32.0            # fp8 storage scale for z = Mh@xn and z2 = Mvo@xn
IZQ = ZS / SM        # PSUM-evac scale for z tensors
IEXP = 1.0 / (32.0 * ZS)   # exp scale: scores psum carries ZS, /sqrt(E)=1/32
W1_FP8 = False       # FFN W1 in fp8 DoubleRow (vs bf16)

_NC_CACHE = {}
LAST_RESULTS = None  # BassKernelResults of the most recent hardware run


def _build_program():
    """Emit the single SPMD program (identical for all 8 cores)."""
    nc = bacc.Bacc(
        "TRN2",
        target_bir_lowering=False,
        debug=False,
        enable_asserts=False,
        num_devices=NCORES,
    )

    d = {}
    d["tgt_t"] = nc.dram_tensor("tgt_t", [4, P, NE, 512], BF16,
                                kind="ExternalInput")
    for w in ("mh1b", "mvo1b"):
        d[w] = nc.dram_tensor(w, [P, NE, E], BF16, kind="ExternalInput")
    d["tgto"] = nc.dram_tensor("tgto", [P, NE, RO], BF16, kind="ExternalInput")
    d["src_t"] = nc.dram_tensor("src_t", [P, NE, S], FP8, kind="ExternalInput")
    d["mask"] = nc.dram_tensor("mask", [P, 8, 512], BF16,
                                kind="ExternalInput")
    for w in ("mh1", "mvo1", "mh2", "mvo2"):
        d[w] = nc.dram_tensor(w, [P, NE, E], FP8, kind="ExternalInput")
    d["w1"] = nc.dram_tensor("w1", [NF, P, NE, P],
                             FP8 if W1_FP8 else BF16, kind="ExternalInput")
    d["w2"] = nc.dram_tensor("w2", [NE, P, NF, P], BF16, kind="ExternalInput")
    d["out_t"] = nc.dram_tensor("out_t", [P, NE * RO], F32, kind="ExternalOutput")

    with tile.TileContext(nc) as tc:
        _emit(tc, {k: v.ap() for k, v in d.items()})

    nc.compile()
    return nc


def _emit(tc, d):
    nc = tc.nc

    def pool(name, bufs=1, side="left"):
        return tc.alloc_tile_pool(name=name, bufs=bufs, side=side)

    # one PSUM pool for the whole program; per-tag bufs; total = 8 banks
    ps = tc.alloc_tile_pool(name="ps", bufs=1, space="PSUM")

    def ps_tile(name, tag, bufs, shape=(P, 512)):
        return ps.tile(list(shape), F32, name=name, tag=tag, bufs=bufs)

    consts = pool("consts")
    ones_8 = consts.tile([P, 2, P], FP8, name="ones_8", tag="ones_8")
    nc.vector.memset(ones_8[:], 1.0)
    ones_b = consts.tile([P, P], BF16, name="ones_b", tag="ones_b")
    nc.vector.memset(ones_b[:], 1.0)
    eps_t = consts.tile([P, 1], F32, name="eps_t", tag="eps")
    nc.vector.memset(eps_t[:], 1e-5)
    # warm up the GpSimd ucode (first tensor-op pays a ~6us IRAM load)
    # during the initial DMA wait
    gwarm = consts.tile([P, 1], BF16, name="gwarm", tag="gwarm")
    nc.gpsimd.tensor_sub(gwarm[:], eps_t[:], eps_t[:])

    tmp = pool("tmp", bufs=1)
    sq_pool = pool("sq", bufs=2)

    def drmm(p_, lhsT, rhs, start, stop):
        nc.tensor.matmul(p_, lhsT, rhs, start=start, stop=stop, perf_mode=DR)

    def _ln_finish(sum_x, sum_xx, stat_pool, prefix, rc, w=512):
        mu = tmp.tile([P, w], F32, name=f"{prefix}mu", tag="t0")
        nc.vector.tensor_scalar_mul(mu[:], sum_x, 1.0 / E)
        musq = tmp.tile([P, w], F32, name=f"{prefix}musq", tag="t1")
        nc.vector.tensor_mul(musq[:], mu[:], mu[:])
        var = tmp.tile([P, w], F32, name=f"{prefix}var", tag="t2")
        nc.vector.scalar_tensor_tensor(
            var[:], sum_xx, 1.0 / E, musq[:], ALU.mult, ALU.subtract)
        std = tmp.tile([P, w], F32, name=f"{prefix}std", tag="t1")
        nc.scalar.activation(std[:], var[:], ACT_F.Sqrt, bias=eps_t[:])
        a32 = tmp.tile([P, w], F32, name=f"{prefix}a32", tag="t2")
        nc.vector.reciprocal_approx_fast(a32[:], std[:])
        # a/bv in bf16: the LN applies then run in the DVE 2x perf mode
        a = stat_pool.tile([P, w], BF16, name=f"{prefix}A{rc}", tag=f"A{rc}")
        nc.scalar.copy(a[:], a32[:])
        bv = stat_pool.tile([P, w], BF16, name=f"{prefix}B{rc}", tag=f"B{rc}")
        nc.vector.tensor_mul(bv[:], mu[:], a32[:])
        return a, bv

    def ln_stats_rc(src_sb, rc, stat_pool, prefix):
        """Per-row LN stats for one 512-row chunk of a bf16 [P, NE, RO]
        transposed activation tensor; sums read the bf16 data directly.
        The squares alternate scalar/vector so neither engine paces the
        stats matmul chain alone."""
        sum_x = ps_tile(f"{prefix}sx{rc}", "sx", 1)
        sum_xx = ps_tile(f"{prefix}sxx{rc}", "sxx", 1)
        for eb in range(NE):
            t = src_sb[:, eb, rc * 512:rc * 512 + 512]
            sq = sq_pool.tile([P, 512], BF16, name=f"{prefix}sq", tag="sqf")
            if eb % 2 == 0:
                nc.scalar.square(sq[:], t)
            else:
                nc.vector.tensor_mul(sq[:], t, t)
            nc.tensor.matmul(sum_x[:], ones_b[:], t,
                             start=(eb == 0), stop=(eb == NE - 1))
            nc.tensor.matmul(sum_xx[:], ones_b[:], sq[:],
                             start=(eb == 0), stop=(eb == NE - 1))
        return _ln_finish(sum_x[:], sum_xx[:], stat_pool, prefix, rc)

    def ln_apply(dst, src_ap, a, bv, prefix, w=512, eb=0):
        """dst (fp8/bf16) = src*A - Bv.  The multiply runs on the DVE in
        its bf16 2x mode; the fp8-writing subtract (which the DVE can only
        do at 1x) alternates onto the otherwise-idle GpSimd engine."""
        t = tmp.tile([P, w], BF16, name=f"{prefix}ap", tag=f"t0{eb % 2}")
        nc.vector.tensor_mul(t[:], src_ap, a[:])
        if eb % 2 == 0:
            nc.vector.tensor_sub(dst, t[:], bv[:])
        else:
            nc.gpsimd.tensor_sub(dst, t[:], bv[:])

    def proj_rc(dst, w_sb, x_sb, rc, tag, scale=IWS):
        """dst[P, NE, rc-chunk] (fp8) = (1/WS) * W.T @ x, DoubleRow fp8."""
        for fblk in range(NE):
            p_ = ps_tile(f"{tag}p", "mm", 2)
            for ep in range(NE // 2):
                drmm(p_[:],
                     w_sb[:, 2 * ep:2 * ep + 2, fblk * P:fblk * P + P],
                     x_sb[:, 2 * ep:2 * ep + 2, rc * 512:rc * 512 + 512],
                     ep == 0, ep == NE // 2 - 1)
            nc.scalar.mul(dst[:, fblk, rc * 512:rc * 512 + 512], p_[:], scale)

    # ------------- long-lived residual tiles (deepest on each side) --------
    h2p = pool("h2p", side="right")
    h2 = h2p.tile([P, NE, RO], BF16, name="h2", tag="h2")
    v2p = pool("v2p", side="right")
    z2c = v2p.tile([P, NKB, E], FP8, name="z2c", tag="v2")
    srcp = pool("srcp", side="right")
    src_sb = srcp.tile([P, NE, S], FP8, name="src_sb", tag="src")
    # (src DMA is emitted later, staggered behind the early-need inputs)

    avtp = pool("avtp", bufs=2)  # AV-evac staging (attn/denominator product)
    q2p = pool("q2p")  # pre-allocated: q2 outlives q1/xnf/z2a on this stack
    q2 = q2p.tile([P, NE, RO], FP8, name="q2", tag="q2")
    q1p = pool("q1p")
    q1 = q1p.tile([P, NE, RO], FP8, name="q1", tag="q1")
    z2ap = pool("z2ap")
    z2a = z2ap.tile([P, NKB, E], FP8, name="z2a", tag="v1")
    xnfp = pool("xnfp")
    xnf = xnfp.tile([P, NE, S], FP8, name="xnf", tag="k1")

    # ------------- LN1 over owned rows -> zq1 ------------------------------
    tgtop = pool("tgtop", side="right")
    tgto_sb = tgtop.tile([P, NE, RO], BF16, name="tgto_sb", tag="tgto")
    for rc in range(2):
        nc.sync.dma_start(tgto_sb[:, :, rc * 512:rc * 512 + 512],
                          d["tgto"][:, :, rc * 512:rc * 512 + 512])
    # resident causal mask: mask[t,j] is t-independent, so one [P,8,512]
    # image serves both halves and the per-kb mask DMAs disappear
    maskp = pool("maskp", side="right")
    mask_sb = maskp.tile([P, 8, 512], BF16, name="mask_sb", tag="mask")
    nc.sync.dma_start(mask_sb[:], d["mask"][:])

    # head-patch staging (the patch itself reuses the LN1o/LN1f stats)
    pp = pool("pp")
    xnh = pp.tile([P, NE, 256], BF16, name="xnh", tag="xnh")
    xnq = pp.tile([P, NE, P], BF16, name="xnq", tag="xnq")
    mvop = pool("mvop")  # one buffer, serially holds mvo1 then mvo2
    l1f = pool("l1f", bufs=2)
    in_pool = pool("inp", bufs=2)
    mvo1 = mvop.tile([P, NE, E], FP8, name="mvo1_sb", tag="mvo")

    l1o = pool("l1o", side="right")
    a1o, b1o = [], []
    for rc in range(2):
        a, bv = ln_stats_rc(tgto_sb, rc, l1o, "l1o")
        a1o.append(a)
        b1o.append(bv)
    # LN1f chunk-0 stats hoisted here: they fill the tensor engine while
    # the DVE runs the LN1o finish + applies
    tins = [in_pool.tile([P, NE, 512], BF16, name="tgt_in0", tag="xin")]
    nc.sync.dma_start(tins[0][:], d["tgt_t"][0])
    af0, bf0 = ln_stats_rc(tins[0], 0, l1f, "l1f")

    mh1p = pool("mh1p", side="right")
    mh1 = mh1p.tile([P, NE, E], FP8, name="mh1_sb", tag="w")
    nc.sync.dma_start(mh1[:], d["mh1"][:])
    xnop = pool("xnop", side="right")
    xno = xnop.tile([P, NE, RO], FP8, name="xno", tag="xno")
    for rc in range(2):
        for eb in range(NE):
            ln_apply(xno[:, eb, rc * 512:rc * 512 + 512],
                     tgto_sb[:, eb, rc * 512:rc * 512 + 512],
                     a1o[rc], b1o[rc], "l1o", eb=eb)
        if rc == 0:  # patch q rows = first owned 128, same stats
            for eb in range(NE):
                ln_apply(xnq[:, eb, :], tgto_sb[:, eb, 0:P],
                         a1o[0][:, 0:P], b1o[0][:, 0:P], "lq", w=P, eb=eb)
        proj_rc(q1, mh1, xno, rc, "q1", scale=IZQ)
    xnop.release()
    mh1p.release()
    l1o.release()
    tins.append(in_pool.tile([P, NE, 512], BF16, name="tgt_in1", tag="xin"))
    nc.sync.dma_start(tins[1][:], d["tgt_t"][1])

    # ------------- LN1 full rows -> xn (scores lhsT) + z2a = (Wv.Wo)@xn ----
    # software-pipelined: chunk rc's z2a matmuls are emitted after chunk
    # rc+1's stats, so the stats fill the tensor engine while the DVE runs
    # chunk rc's LN applies

    def z2a_rc(rc):
        for rb in range(4):
            for vf in range(2):
                vp = ps_tile("vp", "mm", 2)
                for ep in range(NE // 2):
                    drmm(vp[:],
                         xnf[:, 2 * ep:2 * ep + 2,
                             rc * 512 + rb * P:rc * 512 + rb * P + P],
                         mvo1[:, 2 * ep:2 * ep + 2, vf * 512:vf * 512 + 512],
                         ep == 0, ep == NE // 2 - 1)
                nc.scalar.mul(z2a[:, rc * 4 + rb, vf * 512:vf * 512 + 512],
                              vp[:], IZQ)

    # bf16 head patch, folded into the loop as fill: the first owned 128
    # q-rows see <=256 keys, so fp8 errors get no softmax averaging there
    # and would dominate the final error.  Recompute their SA in bf16 with
    # the same composed matrices (zh = Mh@xnq, scores = xnh^T zh,
    # z2h = (Wv.Wo)-composed) and overwrite h1[:, :, 0:128] after SA t0.
    ppw = pool("ppw")
    mh1b = ppw.tile([P, NE, E], BF16, name="mh1b", tag="wqk")
    mvo1b = ppw.tile([P, NE, E], BF16, name="mvo1b", tag="wqk2")
    zh = pp.tile([P, NE, P], BF16, name="zh", tag="qh")
    z2h = pp.tile([P, 2, E], BF16, name="z2h", tag="z2h")
    eth = pp.tile([P, 2, P], BF16, name="eth", tag="eth")
    invh = pp.tile([P, P], F32, name="invh", tag="invh")

    def patch_zh():
        for fb in range(NE):
            qph = ps_tile("qph", "mm", 2, shape=(P, P))
            for eb in range(NE):
                nc.tensor.matmul(qph[:], mh1b[:, eb, fb * P:fb * P + P],
                                 xnq[:, eb, :],
                                 start=(eb == 0), stop=(eb == NE - 1))
            nc.scalar.copy(zh[:, fb, :], qph[:])

    def patch_z2h_scores():
        for kb in range(2):
            for vf in range(2):
                vph = ps_tile("vph", "mm", 2)
                for eb in range(NE):
                    nc.tensor.matmul(
                        vph[:], xnh[:, eb, kb * P:kb * P + P],
                        mvo1b[:, eb, vf * 512:vf * 512 + 512],
                        start=(eb == 0), stop=(eb == NE - 1))
                nc.scalar.copy(z2h[:, kb, vf * 512:vf * 512 + 512], vph[:])
        for kb in range(2):
            sph = ps_tile("sph", "mm", 2, shape=(P, P))
            for eb in range(NE):
                nc.tensor.matmul(sph[:], xnh[:, eb, kb * P:kb * P + P],
                                 zh[:, eb, :],
                                 start=(eb == 0), stop=(eb == NE - 1))
            nc.vector.tensor_add(sph[:], sph[:], mask_sb[:, kb, 0:P])
            nc.scalar.activation(eth[:, kb, :], sph[:], ACT_F.Exp, scale=ISQ)
        smh = ps_tile("smh", "sm", 1, shape=(1, P))
        for kb in range(2):
            nc.tensor.matmul(smh[:], ones_b[:, 0:1], eth[:, kb, :],
                             start=(kb == 0), stop=(kb == 1))
        smhs = pp.tile([1, P], BF16, name="smhs", tag="smhs")
        nc.scalar.copy(smhs[:], smh[:])
        sbh = ps_tile("smbh", "smb", 1, shape=(P, P))
        nc.tensor.matmul(sbh[:], ones_b[0:1, :], smhs[:],
                         start=True, stop=True)
        nc.vector.reciprocal_approx_fast(invh[:], sbh[:])

    for rc in range(4):
        if rc < 2:
            tin, (a1, b1) = tins[rc], ((af0, bf0) if rc == 0 else
                                       ln_stats_rc(tins[1], 0, l1f, "l1f"))
        else:
            tin = in_pool.tile([P, NE, 512], BF16, name="tgt_in", tag="xin")
            nc.sync.dma_start(tin[:], d["tgt_t"][rc])
            a1, b1 = ln_stats_rc(tin, 0, l1f, "l1f")
        for eb in range(NE):
            ln_apply(xnf[:, eb, rc * 512:rc * 512 + 512], tin[:, eb, :],
                     a1, b1, "l1f", eb=eb)
        if rc == 0:  # patch k rows = first 256 full rows, same stats
            for eb in range(NE):
                ln_apply(xnh[:, eb, :], tin[:, eb, 0:256],
                         a1[:, 0:256], b1[:, 0:256], "lh", w=256, eb=eb)
            # big late-use DMAs queue behind the early-need tin chunks
            nc.sync.dma_start(mh1b[:], d["mh1b"][:])
            nc.sync.dma_start(mvo1[:], d["mvo1"][:])
        if rc == 1:
            nc.sync.dma_start(mvo1b[:], d["mvo1b"][:])
            nc.sync.dma_start(src_sb[:], d["src_t"][:])
            patch_zh()
        if rc == 2:
            patch_z2h_scores()
        if rc > 0:
            z2a_rc(rc - 1)
    z2a_rc(3)
    ppw.release()
    in_pool.release()
    l1f.release()


    # ------------- attention helpers (phase-split) -------------------------
    # scores/softmax and AV are emitted separately so independent matmul
    # bursts can be slotted between a phase's serial DVE/scalar chain and
    # the matmuls that consume it
    def attn_scores(q_sb, k_sb, masked, prefix, t):
        ext = (8 * (t + 1)) if masked else NKB
        et = etp.tile([P, NKB, 512], FP8, name=f"{prefix}et{t}", tag="et")
        if masked:
            # diagonal-region blocks only need the upper q-columns; the
            # skipped triangle must be exact zero for the sm/AV sums
            nc.vector.memset(et[:, 0:ext, :], 0.0)
        for kb in range(ext):
            # conservative (parity-independent) fully-masked column count
            cs = P * ((kb - 8 * t) // 2) if masked else 0
            cs = max(0, cs)
            sp = ps_tile(f"{prefix}sp", "av", 2)
            for ep in range(NE // 2):
                drmm(sp[:, cs:512],
                     k_sb[:, 2 * ep:2 * ep + 2, kb * P:kb * P + P],
                     q_sb[:, 2 * ep:2 * ep + 2,
                          t * 512 + cs:t * 512 + 512],
                     ep == 0, ep == NE // 2 - 1)
            if masked and kb >= 8 * t:
                nc.vector.tensor_add(sp[:, cs:512], sp[:, cs:512],
                                     mask_sb[:, kb - 8 * t, cs:512])
            nc.scalar.activation(et[:, kb, cs:512], sp[:, cs:512],
                                 ACT_F.Exp, scale=IEXP)
        # softmax denominator: ones-matmul column sums over key blocks
        sm = ps_tile(f"{prefix}sm", "sm", 1, shape=(1, 512))
        for kp_ in range(ext // 2):
            drmm(sm[:], ones_8[:, :, 0:1], et[:, 2 * kp_:2 * kp_ + 2, :],
                 kp_ == 0, kp_ == ext // 2 - 1)
        sm_sb = etp.tile([1, 512], BF16, name=f"{prefix}smsb", tag="smsb")
        nc.scalar.mul(sm_sb[:], sm[:], ZS)  # fold z2's fp8 scale into 1/den
        sb_ps = ps_tile(f"{prefix}smb", "smb", 1)
        nc.tensor.matmul(sb_ps[:], ones_b[0:1, :], sm_sb[:],
                         start=True, stop=True)
        inv = etp.tile([P, 512], F32, name=f"{prefix}inv{t}", tag="inv")
        nc.vector.reciprocal_approx_fast(inv[:], sb_ps[:])
        return et, inv, ext

    def attn_av(st, v_sb, prefix, t, dst, res_sb):
        """AV with lhsT = z2 = (Wv.Wo)@xn: the PSUM is already the
        O-projection: dst = AV/denominator + residual."""
        et, inv, ext = st
        for af in range(NE):
            ap_ = ps_tile(f"{prefix}avp", "av", 2)
            for kp_ in range(ext // 2):
                drmm(ap_[:],
                     v_sb[:, 2 * kp_:2 * kp_ + 2, af * P:af * P + P],
                     et[:, 2 * kp_:2 * kp_ + 2, :],
                     kp_ == 0, kp_ == ext // 2 - 1)
            t1 = avtp.tile([P, 512], BF16, name=f"{prefix}avt", tag="avt")
            nc.vector.tensor_mul(t1[:], ap_[:], inv[:])
            nc.vector.tensor_add(dst[:, af, t * 512:t * 512 + 512], t1[:],
                                 res_sb[:, af, t * 512:t * 512 + 512])

    etp = pool("etp", side="right")  # attention softmax staging (serial use)
    h1p = pool("h1p", side="right")
    h1 = h1p.tile([P, NE, RO], BF16, name="h1", tag="h1")

    # ------------- SA t0 -> patch -> LN2 rc0 | SA t1 scores | Q2 rc0 | -----
    # SA t1 AV -> LN2 rc1 | CA t0 scores | Q2 rc1 | CA t0 AV -> LN3 rc0 |
    # CA t1 scores | xn3 rc0 -> W1 rc0 | CA t1 AV -> LN3 rc1 | W2 rc0 |
    # W1 rc1 | W2 rc1  -- each serial LN chain is covered by an
    # already-runnable matmul burst emitted right after it
    st0 = attn_scores(q1, xnf, True, "sa", 0)  # covers the mvo2 DMA wait
    # ------------- cross-attention z2c = (Wv2.Wo2)@src (no LN; independent
    # of the whole tgt path -- gap filler for everything up to cross-attn;
    # src_sb itself stays resident as the CA-scores stationary operand)
    mvo2 = mvop.tile([P, NE, E], FP8, name="mvo2_sb", tag="mvo")
    nc.sync.dma_start(mvo2[:], d["mvo2"][:])
    for rc in range(4):
        for rb in range(4):
            for vf in range(2):
                vp = ps_tile("vp2", "mm", 2)
                for ep in range(NE // 2):
                    drmm(vp[:],
                         src_sb[:, 2 * ep:2 * ep + 2,
                                rc * 512 + rb * P:rc * 512 + rb * P + P],
                         mvo2[:, 2 * ep:2 * ep + 2, vf * 512:vf * 512 + 512],
                         ep == 0, ep == NE // 2 - 1)
                nc.scalar.mul(z2c[:, rc * 4 + rb, vf * 512:vf * 512 + 512],
                              vp[:], IZQ)

    attn_av(st0, z2a, "sa", 0, h1, tgto_sb)

    # head patch part B: bf16 AV overwrites h1[:, :, 0:128]
    for af in range(NE):
        avph = ps_tile("avph", "av", 2, shape=(P, P))
        for kb in range(2):
            nc.tensor.matmul(avph[:], z2h[:, kb, af * P:af * P + P],
                             eth[:, kb, :],
                             start=(kb == 0), stop=(kb == 1))
        t1h = avtp.tile([P, P], BF16, name="avth", tag="avth")
        nc.vector.tensor_mul(t1h[:], avph[:], invh[:])
        nc.vector.tensor_add(h1[:, af, 0:P], t1h[:], tgto_sb[:, af, 0:P])
    mvop.release()
    pp.release()

    l2 = pool("l2", side="right")
    a2_0, b2_0 = ln_stats_rc(h1, 0, l2, "l2")
    st1 = attn_scores(q1, xnf, True, "sa", 1)  # fills the LN2-rc0 chain

    mh2p = pool("mh2p", side="right")
    mh2 = mh2p.tile([P, NE, E], FP8, name="mh2_sb", tag="w")
    nc.sync.dma_start(mh2[:], d["mh2"][:])
    xn2p = pool("xn2p", side="right")

    def zq2_rc(rc):
        xn2 = xn2p.tile([P, NE, 512], FP8, name=f"xn2{rc}", tag="xn2")
        a2, b2 = (a2_0, b2_0) if rc == 0 else (a2_1, b2_1)
        for eb in range(NE):
            ln_apply(xn2[:, eb, :], h1[:, eb, rc * 512:rc * 512 + 512],
                     a2, b2, "l2", eb=eb)
        for fblk in range(NE):
            p_ = ps_tile("q2p", "mm", 2)
            for ep in range(NE // 2):
                drmm(p_[:],
                     mh2[:, 2 * ep:2 * ep + 2, fblk * P:fblk * P + P],
                     xn2[:, 2 * ep:2 * ep + 2, :],
                     ep == 0, ep == NE // 2 - 1)
            nc.scalar.mul(q2[:, fblk, rc * 512:rc * 512 + 512], p_[:], IZQ)

    zq2_rc(0)
    attn_av(st1, z2a, "sa", 1, h1, tgto_sb)
    a2_1, b2_1 = ln_stats_rc(h1, 1, l2, "l2")
    stc0 = attn_scores(q2, src_sb, False, "ca", 0)  # fills LN2-rc1 chain
    zq2_rc(1)
    xnfp.release()
    z2ap.release()
    q1p.release()
    xn2p.release()
    mh2p.release()
    l2.release()

    # ------------- CA -> LN3 -> FFN, same interleave -----------------------
    attn_av(stc0, z2c, "ca", 0, h2, h1)

    l3 = pool("l3", side="right")
    xn3p = pool("xn3p", side="right")
    xn3 = xn3p.tile([P, NE, RO], FP8 if W1_FP8 else BF16, name="xn3",
                    tag="xn3")
    a3_0, b3_0 = ln_stats_rc(h2, 0, l3, "l3")
    stc1 = attn_scores(q2, src_sb, False, "ca", 1)  # fills LN3-rc0 chain
    for eb in range(NE):
        ln_apply(xn3[:, eb, 0:512], h2[:, eb, 0:512], a3_0, b3_0, "l3", eb=eb)
    q2p.release()

    hftp = pool("hftp")
    hft = hftp.tile([P, NF, RO], BF16, name="hft", tag="hft")
    w1p = pool("w1p", bufs=3, side="right")

    def w1_rc(rc):
        for fb in range(NF):
            w1t = w1p.tile([P, NE, P], FP8 if W1_FP8 else BF16, name="w1t",
                           tag="w1")
            nc.sync.dma_start(w1t[:], d["w1"][fb])
            hps = ps_tile("hps", "mm", 2)
            if W1_FP8:
                for ep in range(NE // 2):
                    drmm(hps[:], w1t[:, 2 * ep:2 * ep + 2, :],
                         xn3[:, 2 * ep:2 * ep + 2, rc * 512:rc * 512 + 512],
                         ep == 0, ep == NE // 2 - 1)
            else:
                for eb in range(NE):
                    nc.tensor.matmul(
                        hps[:], w1t[:, eb, :],
                        xn3[:, eb, rc * 512:rc * 512 + 512],
                        start=(eb == 0), stop=(eb == NE - 1))
            nc.scalar.activation(hft[:, fb, rc * 512:rc * 512 + 512],
                                 hps[:], ACT_F.Relu,
                                 scale=IWS if W1_FP8 else 1.0)

    attn_av(stc1, z2c, "ca", 1, h2, h1)  # fills the xn3-rc0 applies
    a3_1, b3_1 = ln_stats_rc(h2, 1, l3, "l3")
    w1_rc(0)  # fills the LN3-rc1 chain + xn3-rc1 applies
    for eb in range(NE):
        ln_apply(xn3[:, eb, 512:1024], h2[:, eb, 512:1024], a3_1, b3_1,
                 "l3", eb=eb)
    w1_rc(1)
    w1p.release()
    xn3p.release()
    l3.release()
    h1p.release()
    etp.release()
    maskp.release()
    tgtop.release()
    srcp.release()
    v2p.release()

    outp = pool("outp")
    out_sb = outp.tile([P, NE * RO], F32, name="out_sb", tag="out")
    w2p = pool("w2p", bufs=2, side="right")
    for of in range(NE):
        w2t = w2p.tile([P, NF, P], BF16, name="w2t", tag="w2")
        nc.sync.dma_start(w2t[:], d["w2"][of])
        for rc in range(2):
            ops = ps_tile("ops", "mm", 2)
            for fb in range(NF):
                nc.tensor.matmul(
                    ops[:], w2t[:, fb, :],
                    hft[:, fb, rc * 512:rc * 512 + 512],
                    start=(fb == 0), stop=(fb == NF - 1))
            o = of * RO + rc * 512
            nc.vector.tensor_add(
                out_sb[:, o:o + 512], ops[:],
                h2[:, of, rc * 512:rc * 512 + 512])
        nc.sync.dma_start(d["out_t"][:, of * RO:of * RO + RO],
                          out_sb[:, of * RO:of * RO + RO])
    w2p.release()
    outp.release()
    hftp.release()
    h2p.release()
    avtp.release()
    sq_pool.release()
    tmp.release()
    consts.release()
    ps.release()


# ---------------------------------------------------------------------------
# host side: input swizzling, weight folding, output assembly
# ---------------------------------------------------------------------------

def _to_f8(a):
    return np.clip(a, -240.0, 240.0).astype(F8)


def _swz_w(w_t):
    """[E_in, N] (already [in, out]) -> SBUF image [P, (E_in/P)*N]."""
    e_in, n = w_t.shape
    return np.ascontiguousarray(
        w_t.reshape(e_in // P, P, n).transpose(1, 0, 2).reshape(P, -1))


def _own_rows(h):
    idx = []
    for j in range(8):
        g = 2 * j + h
        idx.extend(range(g * P, (g + 1) * P))
    return np.array(idx)


def make_in_maps(inputs):
    f32 = np.float32
    tgt = np.asarray(inputs["tgt_embs"], f32)
    src = np.asarray(inputs["src_encs"], f32)

    g1 = np.asarray(inputs["ln1_g"], f32)
    g2 = np.asarray(inputs["ln2_g"], f32)
    g3 = np.asarray(inputs["ln3_g"], f32)
    for nm in ("sa_bq", "sa_bk", "sa_bv", "sa_bo", "ca_bq", "ca_bk", "ca_bv",
               "ca_bo", "ff_b1", "ff_b2", "ln1_b", "ln2_b", "ln3_b"):
        assert np.abs(np.asarray(inputs[nm])).max() == 0.0, \
            f"nonzero bias {nm} not supported"

    ws = f32(WS)
    sm = f32(SM)
    wq1 = np.asarray(inputs["sa_Wq"], f32) * g1[None, :]
    wk1 = np.asarray(inputs["sa_Wk"], f32) * g1[None, :]
    wv1 = np.asarray(inputs["sa_Wv"], f32) * g1[None, :]
    wo1 = np.asarray(inputs["sa_Wo"], f32)
    wq2 = np.asarray(inputs["ca_Wq"], f32) * g2[None, :]
    wk2 = np.asarray(inputs["ca_Wk"], f32)
    wv2 = np.asarray(inputs["ca_Wv"], f32)
    wo2 = np.asarray(inputs["ca_Wo"], f32)
    w1 = np.asarray(inputs["ff_W1"], f32) * g3[None, :]
    w2 = np.asarray(inputs["ff_W2"], f32)

    # composed E x E matrices: scores = xn_k^T @ Mh @ xn_q (Mh = Wk^T Wq),
    # attn-out = (Mvo^T-ish @ xn_k)^T @ softmax (Mvo = Wo @ Wv)
    mh1 = wk1.T @ wq1
    mvo1 = wo1 @ wv1
    mh2 = wk2.T @ wq2
    mvo2 = wo2 @ wv2
    w_sb = {
        "mh1": _swz_w(mh1.T * sm).reshape(P, NE, E),
        "mvo1": _swz_w(mvo1.T * sm).reshape(P, NE, E),
        "mh2": _swz_w(mh2.T * sm).reshape(P, NE, E),
        "mvo2": _swz_w(mvo2.T * sm).reshape(P, NE, E),
    }
    w_sb = {k: _to_f8(v) for k, v in w_sb.items()}
    # clean bf16 copies of the composed SA matrices for the head patch
    w_sb["mh1b"] = _swz_w(mh1.T).astype(BF).reshape(P, NE, E)
    w_sb["mvo1b"] = _swz_w(mvo1.T).astype(BF).reshape(P, NE, E)
    w1t = (w1.T * ws) if W1_FP8 else w1.T  # [E, F]
    w1_sw = np.ascontiguousarray(
        w1t.reshape(NE, P, NF, P).transpose(2, 1, 0, 3))
    w1_sw = _to_f8(w1_sw) if W1_FP8 else w1_sw.astype(BF)
    w2t = w2.T.astype(BF)  # [F, E]
    w2_sw = np.ascontiguousarray(
        w2t.reshape(NF, P, NE, P).transpose(2, 1, 0, 3))

    in_maps = []
    for c in range(NCORES):
        b, h = c // 2, c % 2
        rows = _own_rows(h)
        tgt_b_t = tgt[b].T  # [E, S]
        tgt_t = np.ascontiguousarray(
            tgt_b_t.reshape(NE, P, 4, 512).transpose(2, 1, 0, 3)).astype(BF)
        tgto = _swz_w(np.ascontiguousarray(tgt[b][rows].T))
        src_t = _to_f8(_swz_w(src[b].T)).reshape(P, NE, S)
        # mask is identical for both q-row halves (t shifts keys and
        # queries by the same 1024 rows): one [P, 8, 512] resident image
        kr = np.arange(1024)
        qg = np.empty(512, np.int64)
        for s in range(4):
            g = 2 * s + h
            qg[s * P:(s + 1) * P] = g * P + np.arange(P)
        m = np.where(kr[:, None] <= qg[None, :], 0.0, NEG).astype(np.float32)
        mask = np.ascontiguousarray(m.reshape(8, P, 512).transpose(1, 0, 2))
        in_maps.append({
            "tgt_t": tgt_t,
            "tgto": tgto.astype(BF).reshape(P, NE, RO),
            "src_t": src_t,
            "mask": mask.astype(BF),
            **w_sb,
            "w1": w1_sw,
            "w2": w2_sw,
        })
    return in_maps


def assemble_output(results):
    out = np.empty((B, S, E), np.float32)
    for c in range(NCORES):
        b, h = c // 2, c % 2
        arr = np.asarray(results[c]["out_t"])  # [P, NE*RO]
        a = arr.reshape(P, NE, 8, P).transpose(2, 3, 1, 0).reshape(8, P, E)
        for j in range(8):
            g = 2 * j + h
            out[b, g * P:(g + 1) * P, :] = a[j]
    return out


def get_nc():
    if "nc" not in _NC_CACHE:
        _NC_CACHE["nc"] = _build_program()
    return _NC_CACHE["nc"]


def _axon_reset():
    """Recover a wedged remote NeuronCore (NRT_EXEC_UNIT_UNRECOVERABLE)."""
    try:
        import ctypes
        lib = ctypes.CDLL("/opt/axon/libaxon_pjrt.so")
        lib.axon_reset.restype = ctypes.c_int64
        lib.axon_reset()
    except Exception:
        pass


def kernel(**inputs):
    global LAST_RESULTS
    in_maps = make_in_maps(inputs)
    nc = get_nc()
    last_err = None
    for attempt in range(3):
        try:
            res = run_bass_kernel_spmd(nc, in_maps, list(range(NCORES)))
            break
        except Exception as e:  # wedged device -> reset + retry
            last_err = e
            _axon_reset()
    else:
        raise last_err
    LAST_RESULTS = res
    return assemble_output(res.results)

